# revision 27
# baseline (speedup 1.0000x reference)
"""Trainium2 Bass kernel for the DNPU local-receptive-field surrogate model.

Model (see reference): x [B,1,64,64] -> 2x2/stride-2 unfold -> per-node
7-electrode assembly (4 data + 3 control electrodes, placements given by
data_idx/ctrl_idx) -> shared MLP 7->90->(90x4)->1 -> out [B,32,32].

Strategy:
  - Data-parallel over batch: 64 batches per core x 8 cores; tiny weights
    and per-node controls replicated to every core.
  - On-chip layout: hidden units on SBUF partitions, tokens (b,n) on the
    free dim; each matmul processes 512 tokens (one PSUM bank).
  - The whole unfold + electrode scatter + layer-0 contraction is ONE
    K=7 matmul per tile: the host pre-shuffles x into pixel-major layout
    (partition p = patch pixel p, contiguous tokens on the free dim) and
    the per-node controls are broadcast once into partitions 4-6 of the
    same SBUF tile; the stationary operand stacks the gathered W_in rows
    for the 4 data + 3 control electrodes.
  - LAYER-MAJOR sweeps: within a chunk of 8 batches (16 tiles), all
    matmuls of layer L run back-to-back before layer L+1. This keeps the
    PE array continuously streaming (measured: a 512-column matmul
    issues every ~427ns = 1 column/cycle at 1.2GHz, independent of
    dtype; the kernel is PE-throughput-bound at ~768 such streams per
    core) and lets the bias+ReLU PSUM drains trail behind on ACT and
    DVE without gating the PE. GPSIMD cannot read PSUM, and DMA cannot
    touch PSUM, so ACT+DVE are the only drain engines.
  - PSUM pair tiles [90,1024]: two matmuls fill the two bank-aligned
    halves, ONE drain (bias+ReLU fused; ACT activation or DVE
    tensor_scalar add+max, greedily balanced by modeled cost) moves both
    to SBUF, halving drain instruction count and amortizing access
    latency.
  - Output layer: two M=1 matmuls fill the two bank-aligned halves of a
    [1,1024] PSUM tile, retired by one drain; the out matmuls are
    interleaved into the last hidden sweep to smooth engine load.
"""

import ml_dtypes
import numpy as np

import concourse.bass as bass
import concourse.mybir as mybir
import concourse.tile as _tile
from concourse.bass_utils import run_bass_kernel_spmd

# ---------------------------------------------------------------------------
# Workaround: this neuronxcc walrus build rejects instructions carrying more
# than a couple of sem waits ("Too many sync wait commands"). Tile freely
# attaches several waits to one instruction (and its tail drain waits on
# every proc sem at once). After scheduling, spill excess waits onto NOPs
# inserted just before the instruction on the same engine — engines execute
# their stream in order, so semantics are unchanged.
_MAX_SYNC_WAITS = 1
_nop_counter = [0]


def _split_excess_sync_waits(nc, maxw=_MAX_SYNC_WAITS):
    for f in nc.m.functions:
        for bb in f.blocks:
            insts = list(bb.instructions)
            if not any(
                ins.sync_info is not None and len(ins.sync_info.on_wait or []) > maxw
                for ins in insts
            ):
                continue
            new = []
            for ins in insts:
                si = ins.sync_info
                waits = list(si.on_wait or []) if si is not None else []
                if len(waits) > maxw:
                    excess, keep = waits[: len(waits) - maxw], waits[-maxw:]
                    for i in range(0, len(excess), maxw):
                        _nop_counter[0] += 1
                        nop = mybir.InstNoOp(name=f"waitsplit_{_nop_counter[0]}")
                        nop.engine = ins.engine
                        nop.sync_info = mybir.SyncInfo(
                            on_wait=excess[i : i + maxw], on_update=[]
                        )
                        new.append(nop)
                    si.on_wait = keep
                new.append(ins)
            bb.instructions = new

# ---------------------------------------------------------------------------
# Problem constants (hardcoded per the task contract).
B = 512
H = W = 64
K = 2
N_NODES = (H // K) * (W // K)  # 1024
HID = 90
N_HIDDEN = 4
N_CORES = 8
B_CORE = B // N_CORES  # 64 batches per core

CHUNK_B = 8  # batches per layer-major chunk
N_TILE = 512  # tokens per matmul (one PSUM bank of fp32)

F32 = mybir.dt.float32
BF16 = mybir.dt.bfloat16  # matmul operand dtype (PSUM accum stays fp32)

# modeled drain costs (ns) for the greedy ACT/DVE balancer
_COST = {
    "act_pair": 1104.0,
    "dve_pair": 1276.0,
    "act_out": 602.0,
    "dve_out": 660.0,
}


def _build_program(b_core: int, chunk_b: int):
    """Trace the per-core Bass program (identical on all 8 cores)."""
    nc = bass.Bass()

    # host-preshuffled x: [4, b_core*1024]; partition p = patch pixel p
    xs_d = nc.dram_tensor("xs", [4, b_core * N_NODES], BF16, kind="ExternalInput")
    # controls contribution rows tiled to one chunk: [3, chunk_b*1024]
    ctl_d = nc.dram_tensor("ctl", [3, chunk_b * N_NODES], BF16, kind="ExternalInput")
    wz_d = nc.dram_tensor("wz", [7, HID], BF16, kind="ExternalInput")
    wh_d = nc.dram_tensor("wh", [HID, N_HIDDEN, HID], BF16, kind="ExternalInput")
    wo_d = nc.dram_tensor("wo", [HID, 1], BF16, kind="ExternalInput")
    bia_d = nc.dram_tensor("bia", [HID, 5], F32, kind="ExternalInput")
    bo_d = nc.dram_tensor("bo", [1], F32, kind="ExternalInput")
    out_d = nc.dram_tensor("out", [b_core, N_NODES], F32, kind="ExternalOutput")

    n_chunks = b_core // chunk_b
    chunk_tok = chunk_b * N_NODES
    tiles = chunk_tok // N_TILE  # matmul tiles per layer sweep (16)
    pairs = tiles // 2
    quads = tiles // 4

    Relu = mybir.ActivationFunctionType.Relu
    Identity = mybir.ActivationFunctionType.Identity
    ALU_ADD = mybir.AluOpType.add
    ALU_MAX = mybir.AluOpType.max

    eng_t = [0.0, 0.0]  # modeled busy ns: [ACT, DVE]

    with _tile.TileContext(nc) as tc:
        with (
            tc.tile_pool(name="const", bufs=1) as const,
            tc.tile_pool(name="xin", bufs=1) as xin,
            tc.tile_pool(name="outp", bufs=2) as outp,
            tc.tile_pool(name="hbuf", bufs=2 * pairs) as hbuf,
            tc.tile_pool(name="ps", bufs=2, space="PSUM") as ps,
            tc.tile_pool(name="pso", bufs=4, space="PSUM") as pso,
        ):
            # ---- constants ----
            # the first matmul needs x chunk 0 + wz + ctrl rows: issue those
            # DMAs first so they are never queued behind the bulky consts.
            xt = xin.tile([7, chunk_tok], BF16, tag="xt")
            nc.sync.dma_start(
                xt[0:4, :], xs_d[:, 0:chunk_tok]
            )
            wz = const.tile([7, HID], BF16)
            nc.sync.dma_start(wz[:], wz_d[:])
            nc.sync.dma_start(xt[4:7, :], ctl_d[:])
            bia = const.tile([HID, 5], F32)
            nc.sync.dma_start(bia[:], bia_d[:])
            bin_t = bia  # column 0 = b_in, columns 1..4 = b_h
            wh = const.tile([HID, N_HIDDEN, HID], BF16)
            nc.sync.dma_start(wh[:], wh_d[:])
            wo = const.tile([HID, 1], BF16)
            nc.sync.dma_start(wo[:], wo_d[:])
            bo_t = const.tile([1, 1], F32)
            nc.sync.dma_start(bo_t[:], bo_d[:].unsqueeze(0))

            def drain(dst, src, bias_ap, relu, kind, eng=None):
                """PSUM->SBUF bias+(relu). eng forces ACT(0)/DVE(1); default
                picks the modeled-least-busy engine."""
                ca, cd = _COST[f"act_{kind}"], _COST[f"dve_{kind}"]
                if eng is None:
                    use_act = eng_t[0] + ca <= eng_t[1] + cd
                else:
                    use_act = eng == 0
                if use_act:
                    eng_t[0] += ca
                    nc.scalar.activation(
                        dst, src, Relu if relu else Identity, bias=bias_ap
                    )
                elif relu:
                    eng_t[1] += cd
                    nc.vector.tensor_scalar(
                        out=dst,
                        in0=src,
                        scalar1=bias_ap,
                        scalar2=0.0,
                        op0=ALU_ADD,
                        op1=ALU_MAX,
                    )
                else:
                    eng_t[1] += cd
                    nc.vector.tensor_scalar(
                        out=dst,
                        in0=src,
                        scalar1=bias_ap,
                        scalar2=None,
                        op0=ALU_ADD,
                    )

            for ck in range(n_chunks):
                b0 = ck * chunk_b
                t0 = b0 * N_NODES
                if ck > 0:
                    nc.sync.dma_start(
                        xt[0:4, :], xs_d[:, t0 : t0 + chunk_tok]
                    )

                # ---- layer 0 sweep: one K=7 matmul per tile
                hs = []
                for pr in range(pairs):
                    pt = ps.tile([HID, 2 * N_TILE], F32, tag="ps")
                    for half in range(2):
                        c0 = (2 * pr + half) * N_TILE
                        nc.tensor.matmul(
                            pt[:, half * N_TILE : (half + 1) * N_TILE],
                            wz[:],
                            xt[:, c0 : c0 + N_TILE],
                        )
                    h = hbuf.tile([HID, 2 * N_TILE], BF16, tag="h")
                    drain(h[:], pt[:], bia[:, 0:1], relu=True, kind="pair")
                    hs.append(h)

                # ---- hidden layer sweeps (out matmuls interleave into l3)
                for li in range(N_HIDDEN):
                    last = li == N_HIDDEN - 1
                    if last:
                        o = outp.tile([1, chunk_tok], F32, tag="o")
                        emitted = set()
                    hs2 = []
                    for pr in range(pairs):
                        pt = ps.tile([HID, 2 * N_TILE], F32, tag="ps")
                        for half in range(2):
                            nc.tensor.matmul(
                                pt[:, half * N_TILE : (half + 1) * N_TILE],
                                wh[:, li, :],
                                hs[pr][:, half * N_TILE : (half + 1) * N_TILE],
                            )
                        h = hbuf.tile([HID, 2 * N_TILE], BF16, tag="h")
                        drain(
                            h[:],
                            pt[:],
                            bia[:, li + 1 : li + 2],
                            relu=True,
                            kind="pair",
                        )
                        hs2.append(h)
                        if last and pr >= 2:
                            # out matmuls for the pair drained 2 pairs ago
                            _emit_out_pair(
                                nc, pso, hs2, o, bo_t, wo, pr - 2, drain
                            )
                            emitted.add(pr - 2)
                    hs = hs2
                    if last:
                        for pr in range(pairs):
                            if pr not in emitted:
                                _emit_out_pair(nc, pso, hs, o, bo_t, wo, pr, drain)

                nc.sync.dma_start(
                    out_d[b0 : b0 + chunk_b].rearrange("b n -> (b n)").unsqueeze(0),
                    o[:],
                )

    _split_excess_sync_waits(nc)
    return nc


def _emit_out_pair(nc, pso, hs, o, bo_t, wo, pr, drain, eng=None):
    """2 out matmuls (M=1), each into its own [1,512] PSUM tile with its
    own drain (2-deep ring keeps the PE from waiting on out drains)."""
    for half in range(2):
        pt = pso.tile([1, N_TILE], mybir.dt.float32, tag="po")
        nc.tensor.matmul(
            pt[:],
            wo[:],
            hs[pr][:, half * N_TILE : (half + 1) * N_TILE],
        )
        t = 2 * pr + half
        drain(
            o[0:1, t * N_TILE : (t + 1) * N_TILE],
            pt[:],
            bo_t[0:1, 0:1],
            relu=False,
            kind="out",
            eng=eng,
        )


def _prep_weights(controls, W_in, b_in, W_h, b_h, W_out, b_out, data_idx, ctrl_idx):
    """Host-side prep: gather W_in rows per electrode placement (replicating
    the reference's scatter semantics) and tile the control matrix to one
    chunk."""
    di = np.asarray(data_idx)[0].tolist()  # placements identical across nodes
    ci = np.asarray(ctrl_idx)[0].tolist()
    W_in = np.asarray(W_in, dtype=np.float32)
    Wd = W_in[di, :].copy()  # [4, HID]
    cset = set(ci)
    for j in range(4):
        if di[j] in cset or di[j] in di[j + 1 :]:
            Wd[j] = 0.0  # overwritten by a control (or a later data) electrode
    Wc = W_in[ci, :].copy()  # [3, HID]
    for k in range(3):
        if ci[k] in ci[k + 1 :]:
            Wc[k] = 0.0  # later control write wins

    bf = ml_dtypes.bfloat16
    ctl = np.ascontiguousarray(
        np.tile(np.asarray(controls, np.float32).T, (1, CHUNK_B)).astype(bf)
    )  # [3, chunk_tok]

    common = {
        "ctl": ctl,
        "wz": np.ascontiguousarray(
            np.concatenate([Wd, Wc], axis=0).astype(bf)
        ),  # [7, HID]
        "wh": np.ascontiguousarray(
            np.asarray(W_h, np.float32).astype(bf).transpose(1, 0, 2)
        ),
        "wo": np.ascontiguousarray(np.asarray(W_out, np.float32).astype(bf)),
        "bia": np.ascontiguousarray(
            np.concatenate(
                [np.asarray(b_in, np.float32)[:, None],
                 np.asarray(b_h, np.float32).T],
                axis=1,
            )
        ),
        "bo": np.ascontiguousarray(np.asarray(b_out, np.float32)),
    }
    return common


def _shuffle_x(x_core):
    """[b,64,64] -> [4, b*1024]: partition p=(kh*2+kw), tokens (b, node)."""
    b = x_core.shape[0]
    p = x_core.reshape(b, 32, 2, 32, 2).transpose(2, 4, 0, 1, 3)
    return np.ascontiguousarray(
        p.reshape(4, b * N_NODES).astype(ml_dtypes.bfloat16)
    )


def _run(inputs, trace=False, tmpdir=None):
    x = np.asarray(inputs["x"], dtype=np.float32)
    common = _prep_weights(
        inputs["controls"],
        inputs["W_in"],
        inputs["b_in"],
        inputs["W_h"],
        inputs["b_h"],
        inputs["W_out"],
        inputs["b_out"],
        inputs["data_idx"],
        inputs["ctrl_idx"],
    )

    nc = _build_program(B_CORE, CHUNK_B)

    core_ids = list(range(N_CORES))
    in_maps = []
    for i in core_ids:
        shard = _shuffle_x(x[i * B_CORE : (i + 1) * B_CORE, 0])
        in_maps.append({"xs": shard, **common})

    res = run_bass_kernel_spmd(nc, in_maps, core_ids, trace=trace, tmpdir=tmpdir)
    out = np.concatenate([res.results[i]["out"] for i in core_ids], axis=0)
    return out.reshape(B, 32, 32), res.exec_time_ns


def kernel(**inputs):
    return _run(inputs, trace=False)[0]


# revision 28
# speedup vs baseline: 1.2321x; 1.2321x over previous
"""Trainium2 Bass kernel for the DNPU local-receptive-field surrogate model.

Model (see reference): x [B,1,64,64] -> 2x2/stride-2 unfold -> per-node
7-electrode assembly (4 data + 3 control electrodes, placements given by
data_idx/ctrl_idx) -> shared MLP 7->90->(90x4)->1 -> out [B,32,32].

Strategy:
  - Data-parallel over batch: 64 batches per core x 8 cores; tiny weights
    and per-node controls replicated to every core.
  - On-chip layout: hidden units on SBUF partitions, tokens (b,n) on the
    free dim; each matmul processes 512 tokens (one PSUM bank).
  - The whole unfold + electrode scatter + layer-0 contraction is ONE
    K=7 matmul per tile: the host pre-shuffles x into pixel-major layout
    (partition p = patch pixel p, contiguous tokens on the free dim) and
    the per-node controls are broadcast once into partitions 4-6 of the
    same SBUF tile; the stationary operand stacks the gathered W_in rows
    for the 4 data + 3 control electrodes.
  - LAYER-MAJOR sweeps: within a chunk of 8 batches (16 tiles), all
    matmuls of layer L run back-to-back before layer L+1. This keeps the
    PE array continuously streaming (measured: a 512-column matmul
    issues every ~427ns = 1 column/cycle at 1.2GHz, independent of
    dtype; the kernel is PE-throughput-bound at ~768 such streams per
    core) and lets the bias+ReLU PSUM drains trail behind on ACT and
    DVE without gating the PE. GPSIMD cannot read PSUM, and DMA cannot
    touch PSUM, so ACT+DVE are the only drain engines.
  - PSUM pair tiles [90,1024]: two matmuls fill the two bank-aligned
    halves, ONE drain (bias+ReLU fused; ACT activation or DVE
    tensor_scalar add+max, greedily balanced by modeled cost) moves both
    to SBUF, halving drain instruction count and amortizing access
    latency.
  - Output layer: two M=1 matmuls fill the two bank-aligned halves of a
    [1,1024] PSUM tile, retired by one drain; the out matmuls are
    interleaved into the last hidden sweep to smooth engine load.
"""

import ml_dtypes
import numpy as np

import concourse.bass as bass
import concourse.mybir as mybir
import concourse.tile as _tile
from concourse.bass_utils import run_bass_kernel_spmd

# ---------------------------------------------------------------------------
# Workaround: this neuronxcc walrus build rejects instructions carrying more
# than a couple of sem waits ("Too many sync wait commands"). Tile freely
# attaches several waits to one instruction (and its tail drain waits on
# every proc sem at once). After scheduling, spill excess waits onto NOPs
# inserted just before the instruction on the same engine — engines execute
# their stream in order, so semantics are unchanged.
_MAX_SYNC_WAITS = 1
_nop_counter = [0]


def _split_excess_sync_waits(nc, maxw=_MAX_SYNC_WAITS):
    for f in nc.m.functions:
        for bb in f.blocks:
            insts = list(bb.instructions)
            if not any(
                ins.sync_info is not None and len(ins.sync_info.on_wait or []) > maxw
                for ins in insts
            ):
                continue
            new = []
            for ins in insts:
                si = ins.sync_info
                waits = list(si.on_wait or []) if si is not None else []
                if len(waits) > maxw:
                    excess, keep = waits[: len(waits) - maxw], waits[-maxw:]
                    for i in range(0, len(excess), maxw):
                        _nop_counter[0] += 1
                        nop = mybir.InstNoOp(name=f"waitsplit_{_nop_counter[0]}")
                        nop.engine = ins.engine
                        nop.sync_info = mybir.SyncInfo(
                            on_wait=excess[i : i + maxw], on_update=[]
                        )
                        new.append(nop)
                    si.on_wait = keep
                new.append(ins)
            bb.instructions = new

# ---------------------------------------------------------------------------
# Problem constants (hardcoded per the task contract).
B = 512
H = W = 64
K = 2
N_NODES = (H // K) * (W // K)  # 1024
HID = 90
N_HIDDEN = 4
N_CORES = 8
B_CORE = B // N_CORES  # 64 batches per core

CHUNK_B = 8  # batches per layer-major chunk
N_TILE = 512  # tokens per matmul (one PSUM bank of fp32)

F32 = mybir.dt.float32
BF16 = mybir.dt.bfloat16  # matmul operand dtype (PSUM accum stays fp32)

# modeled drain costs (ns) for the greedy ACT/DVE balancer
_COST = {
    "act_pair": 1104.0,
    "dve_pair": 1276.0,
    "act_out": 602.0,
    "dve_out": 660.0,
}


def _build_program(b_core: int, chunk_b: int):
    """Trace the per-core Bass program (identical on all 8 cores)."""
    nc = bass.Bass()

    # host-preshuffled x: [4, b_core*1024]; partition p = patch pixel p
    xs_d = nc.dram_tensor("xs", [4, b_core * N_NODES], BF16, kind="ExternalInput")
    # controls contribution rows tiled to one chunk: [3, chunk_b*1024]
    ctl_d = nc.dram_tensor("ctl", [3, chunk_b * N_NODES], BF16, kind="ExternalInput")
    wz_d = nc.dram_tensor("wz", [7, HID], BF16, kind="ExternalInput")
    wh_d = nc.dram_tensor("wh", [HID, N_HIDDEN, HID], BF16, kind="ExternalInput")
    wo_d = nc.dram_tensor("wo", [HID, 1], BF16, kind="ExternalInput")
    bia_d = nc.dram_tensor("bia", [HID, 5], F32, kind="ExternalInput")
    bo_d = nc.dram_tensor("bo", [1], F32, kind="ExternalInput")
    out_d = nc.dram_tensor("out", [b_core, N_NODES], F32, kind="ExternalOutput")

    n_chunks = b_core // chunk_b
    chunk_tok = chunk_b * N_NODES
    tiles = chunk_tok // N_TILE  # matmul tiles per layer sweep (16)
    pairs = tiles // 2
    quads = tiles // 4

    Relu = mybir.ActivationFunctionType.Relu
    Identity = mybir.ActivationFunctionType.Identity
    ALU_ADD = mybir.AluOpType.add
    ALU_MAX = mybir.AluOpType.max

    eng_t = [0.0, 0.0]  # modeled busy ns: [ACT, DVE]

    with _tile.TileContext(nc) as tc:
        with (
            tc.tile_pool(name="const", bufs=1) as const,
            tc.tile_pool(name="xin", bufs=1) as xin,
            tc.tile_pool(name="outp", bufs=2) as outp,
            tc.tile_pool(name="hbuf", bufs=2 * pairs) as hbuf,
            tc.tile_pool(name="ps", bufs=3, space="PSUM") as ps,
            tc.tile_pool(name="pso", bufs=2, space="PSUM") as pso,
        ):
            # ---- constants ----
            # the first matmul needs x chunk 0 + wz + ctrl rows: issue those
            # DMAs first so they are never queued behind the bulky consts.
            xt = xin.tile([7, chunk_tok], BF16, tag="xt")
            nc.sync.dma_start(
                xt[0:4, :], xs_d[:, 0:chunk_tok]
            )
            wz = const.tile([7, HID], BF16)
            nc.sync.dma_start(wz[:], wz_d[:])
            nc.sync.dma_start(xt[4:7, :], ctl_d[:])
            bia = const.tile([HID, 5], F32)
            nc.sync.dma_start(bia[:], bia_d[:])
            bin_t = bia  # column 0 = b_in, columns 1..4 = b_h
            wh = const.tile([HID, N_HIDDEN, HID], BF16)
            nc.sync.dma_start(wh[:], wh_d[:])
            wo = const.tile([HID, 1], BF16)
            nc.sync.dma_start(wo[:], wo_d[:])
            bo_t = const.tile([1, 1], F32)
            nc.sync.dma_start(bo_t[:], bo_d[:].unsqueeze(0))

            def drain(dst, src, bias_ap, relu, kind, eng=None):
                """PSUM->SBUF bias+(relu). eng forces ACT(0)/DVE(1); default
                picks the modeled-least-busy engine."""
                ca, cd = _COST[f"act_{kind}"], _COST[f"dve_{kind}"]
                if eng is None:
                    use_act = eng_t[0] + ca <= eng_t[1] + cd
                else:
                    use_act = eng == 0
                if use_act:
                    eng_t[0] += ca
                    nc.scalar.activation(
                        dst, src, Relu if relu else Identity, bias=bias_ap
                    )
                elif relu:
                    eng_t[1] += cd
                    nc.vector.tensor_scalar(
                        out=dst,
                        in0=src,
                        scalar1=bias_ap,
                        scalar2=0.0,
                        op0=ALU_ADD,
                        op1=ALU_MAX,
                    )
                else:
                    eng_t[1] += cd
                    nc.vector.tensor_scalar(
                        out=dst,
                        in0=src,
                        scalar1=bias_ap,
                        scalar2=None,
                        op0=ALU_ADD,
                    )

            for ck in range(n_chunks):
                b0 = ck * chunk_b
                t0 = b0 * N_NODES
                if ck > 0:
                    nc.sync.dma_start(
                        xt[0:4, :], xs_d[:, t0 : t0 + chunk_tok]
                    )

                # ---- layer 0 sweep: one K=7 matmul per tile
                hs = []
                for pr in range(pairs):
                    pt = ps.tile([HID, 2 * N_TILE], F32, tag="ps")
                    for half in range(2):
                        c0 = (2 * pr + half) * N_TILE
                        nc.tensor.matmul(
                            pt[:, half * N_TILE : (half + 1) * N_TILE],
                            wz[:],
                            xt[:, c0 : c0 + N_TILE],
                        )
                    h = hbuf.tile([HID, 2 * N_TILE], BF16, tag="h")
                    drain(h[:], pt[:], bia[:, 0:1], relu=True, kind="pair")
                    hs.append(h)

                # ---- hidden layer sweeps (out matmuls interleave into l3)
                for li in range(N_HIDDEN):
                    last = li == N_HIDDEN - 1
                    if last:
                        o = outp.tile([1, chunk_tok], F32, tag="o")
                        emitted = set()
                    hs2 = []
                    for pr in range(pairs):
                        pt = ps.tile([HID, 2 * N_TILE], F32, tag="ps")
                        for half in range(2):
                            nc.tensor.matmul(
                                pt[:, half * N_TILE : (half + 1) * N_TILE],
                                wh[:, li, :],
                                hs[pr][:, half * N_TILE : (half + 1) * N_TILE],
                            )
                        h = hbuf.tile([HID, 2 * N_TILE], BF16, tag="h")
                        drain(
                            h[:],
                            pt[:],
                            bia[:, li + 1 : li + 2],
                            relu=True,
                            kind="pair",
                        )
                        hs2.append(h)
                        if last and pr >= 2:
                            # out matmuls for the pair drained 2 pairs ago
                            _emit_out_pair(
                                nc, pso, hs2, o, bo_t, wo, pr - 2, drain
                            )
                            emitted.add(pr - 2)
                    hs = hs2
                    if last:
                        for pr in range(pairs):
                            if pr not in emitted:
                                _emit_out_pair(nc, pso, hs, o, bo_t, wo, pr, drain)

                nc.sync.dma_start(
                    out_d[b0 : b0 + chunk_b].rearrange("b n -> (b n)").unsqueeze(0),
                    o[:],
                )

    _split_excess_sync_waits(nc)
    return nc


def _emit_out_pair(nc, pso, hs, o, bo_t, wo, pr, drain, eng=None):
    """2 out matmuls (M=1), each into its own [1,512] PSUM tile with its
    own drain (2-deep ring keeps the PE from waiting on out drains)."""
    for half in range(2):
        pt = pso.tile([1, N_TILE], mybir.dt.float32, tag="po")
        nc.tensor.matmul(
            pt[:],
            wo[:],
            hs[pr][:, half * N_TILE : (half + 1) * N_TILE],
        )
        t = 2 * pr + half
        drain(
            o[0:1, t * N_TILE : (t + 1) * N_TILE],
            pt[:],
            bo_t[0:1, 0:1],
            relu=False,
            kind="out",
            eng=eng,
        )


def _prep_weights(controls, W_in, b_in, W_h, b_h, W_out, b_out, data_idx, ctrl_idx):
    """Host-side prep: gather W_in rows per electrode placement (replicating
    the reference's scatter semantics) and tile the control matrix to one
    chunk."""
    di = np.asarray(data_idx)[0].tolist()  # placements identical across nodes
    ci = np.asarray(ctrl_idx)[0].tolist()
    W_in = np.asarray(W_in, dtype=np.float32)
    Wd = W_in[di, :].copy()  # [4, HID]
    cset = set(ci)
    for j in range(4):
        if di[j] in cset or di[j] in di[j + 1 :]:
            Wd[j] = 0.0  # overwritten by a control (or a later data) electrode
    Wc = W_in[ci, :].copy()  # [3, HID]
    for k in range(3):
        if ci[k] in ci[k + 1 :]:
            Wc[k] = 0.0  # later control write wins

    bf = ml_dtypes.bfloat16
    ctl = np.ascontiguousarray(
        np.tile(np.asarray(controls, np.float32).T, (1, CHUNK_B)).astype(bf)
    )  # [3, chunk_tok]

    common = {
        "ctl": ctl,
        "wz": np.ascontiguousarray(
            np.concatenate([Wd, Wc], axis=0).astype(bf)
        ),  # [7, HID]
        "wh": np.ascontiguousarray(
            np.asarray(W_h, np.float32).astype(bf).transpose(1, 0, 2)
        ),
        "wo": np.ascontiguousarray(np.asarray(W_out, np.float32).astype(bf)),
        "bia": np.ascontiguousarray(
            np.concatenate(
                [np.asarray(b_in, np.float32)[:, None],
                 np.asarray(b_h, np.float32).T],
                axis=1,
            )
        ),
        "bo": np.ascontiguousarray(np.asarray(b_out, np.float32)),
    }
    return common


def _shuffle_x(x_core):
    """[b,64,64] -> [4, b*1024]: partition p=(kh*2+kw), tokens (b, node)."""
    b = x_core.shape[0]
    p = x_core.reshape(b, 32, 2, 32, 2).transpose(2, 4, 0, 1, 3)
    return np.ascontiguousarray(
        p.reshape(4, b * N_NODES).astype(ml_dtypes.bfloat16)
    )


def _run(inputs, trace=False, tmpdir=None):
    x = np.asarray(inputs["x"], dtype=np.float32)
    common = _prep_weights(
        inputs["controls"],
        inputs["W_in"],
        inputs["b_in"],
        inputs["W_h"],
        inputs["b_h"],
        inputs["W_out"],
        inputs["b_out"],
        inputs["data_idx"],
        inputs["ctrl_idx"],
    )

    nc = _build_program(B_CORE, CHUNK_B)

    core_ids = list(range(N_CORES))
    in_maps = []
    for i in core_ids:
        shard = _shuffle_x(x[i * B_CORE : (i + 1) * B_CORE, 0])
        in_maps.append({"xs": shard, **common})

    res = run_bass_kernel_spmd(nc, in_maps, core_ids, trace=trace, tmpdir=tmpdir)
    out = np.concatenate([res.results[i]["out"] for i in core_ids], axis=0)
    return out.reshape(B, 32, 32), res.exec_time_ns


def kernel(**inputs):
    return _run(inputs, trace=False)[0]


# revision 29
# speedup vs baseline: 1.2692x; 1.0301x over previous
"""Trainium2 Bass kernel for the DNPU local-receptive-field surrogate model.

Model (see reference): x [B,1,64,64] -> 2x2/stride-2 unfold -> per-node
7-electrode assembly (4 data + 3 control electrodes, placements given by
data_idx/ctrl_idx) -> shared MLP 7->90->(90x4)->1 -> out [B,32,32].

Strategy:
  - Data-parallel over batch: 64 batches per core x 8 cores; tiny weights
    and per-node controls replicated to every core.
  - On-chip layout: hidden units on SBUF partitions, tokens (b,n) on the
    free dim; each matmul processes 512 tokens (one PSUM bank).
  - The whole unfold + electrode scatter + layer-0 contraction is ONE
    K=7 matmul per tile: the host pre-shuffles x into pixel-major layout
    (partition p = patch pixel p, contiguous tokens on the free dim) and
    the per-node controls are broadcast once into partitions 4-6 of the
    same SBUF tile; the stationary operand stacks the gathered W_in rows
    for the 4 data + 3 control electrodes.
  - LAYER-MAJOR sweeps: within a chunk of 8 batches (16 tiles), all
    matmuls of layer L run back-to-back before layer L+1. This keeps the
    PE array continuously streaming (measured: a 512-column matmul
    issues every ~427ns = 1 column/cycle at 1.2GHz, independent of
    dtype; the kernel is PE-throughput-bound at ~768 such streams per
    core) and lets the bias+ReLU PSUM drains trail behind on ACT and
    DVE without gating the PE. GPSIMD cannot read PSUM, and DMA cannot
    touch PSUM, so ACT+DVE are the only drain engines.
  - PSUM pair tiles [90,1024]: two matmuls fill the two bank-aligned
    halves, ONE drain (bias+ReLU fused; ACT activation or DVE
    tensor_scalar add+max, greedily balanced by modeled cost) moves both
    to SBUF, halving drain instruction count and amortizing access
    latency.
  - Output layer: two M=1 matmuls fill the two bank-aligned halves of a
    [1,1024] PSUM tile, retired by one drain; the out matmuls are
    interleaved into the last hidden sweep to smooth engine load.
"""

import ml_dtypes
import numpy as np

import concourse.bass as bass
import concourse.mybir as mybir
import concourse.tile as _tile
from concourse.bass_utils import run_bass_kernel_spmd

# ---------------------------------------------------------------------------
# Workaround: this neuronxcc walrus build rejects instructions carrying more
# than a couple of sem waits ("Too many sync wait commands"). Tile freely
# attaches several waits to one instruction (and its tail drain waits on
# every proc sem at once). After scheduling, spill excess waits onto NOPs
# inserted just before the instruction on the same engine — engines execute
# their stream in order, so semantics are unchanged.
_MAX_SYNC_WAITS = 1
_nop_counter = [0]


def _split_excess_sync_waits(nc, maxw=_MAX_SYNC_WAITS):
    for f in nc.m.functions:
        for bb in f.blocks:
            insts = list(bb.instructions)
            if not any(
                ins.sync_info is not None and len(ins.sync_info.on_wait or []) > maxw
                for ins in insts
            ):
                continue
            new = []
            for ins in insts:
                si = ins.sync_info
                waits = list(si.on_wait or []) if si is not None else []
                if len(waits) > maxw:
                    excess, keep = waits[: len(waits) - maxw], waits[-maxw:]
                    for i in range(0, len(excess), maxw):
                        _nop_counter[0] += 1
                        nop = mybir.InstNoOp(name=f"waitsplit_{_nop_counter[0]}")
                        nop.engine = ins.engine
                        nop.sync_info = mybir.SyncInfo(
                            on_wait=excess[i : i + maxw], on_update=[]
                        )
                        new.append(nop)
                    si.on_wait = keep
                new.append(ins)
            bb.instructions = new

# ---------------------------------------------------------------------------
# Problem constants (hardcoded per the task contract).
B = 512
H = W = 64
K = 2
N_NODES = (H // K) * (W // K)  # 1024
HID = 90
N_HIDDEN = 4
N_CORES = 8
B_CORE = B // N_CORES  # 64 batches per core

CHUNK_B = 8  # batches per layer-major chunk
N_TILE = 512  # tokens per matmul (one PSUM bank of fp32)

F32 = mybir.dt.float32
BF16 = mybir.dt.bfloat16  # matmul operand dtype (PSUM accum stays fp32)

# modeled drain costs (ns) for the greedy ACT/DVE balancer
_COST = {
    "act_pair": 1104.0,
    "dve_pair": 1276.0,
    "act_out": 602.0,
    "dve_out": 660.0,
}


def _build_program(b_core: int, chunk_b: int):
    """Trace the per-core Bass program (identical on all 8 cores)."""
    nc = bass.Bass()

    # host-preshuffled x: [4, b_core*1024]; partition p = patch pixel p
    xs_d = nc.dram_tensor("xs", [4, b_core * N_NODES], BF16, kind="ExternalInput")
    # controls contribution rows tiled to one chunk: [3, chunk_b*1024]
    ctl_d = nc.dram_tensor("ctl", [3, chunk_b * N_NODES], BF16, kind="ExternalInput")
    wz_d = nc.dram_tensor("wz", [7, HID], BF16, kind="ExternalInput")
    wh_d = nc.dram_tensor("wh", [HID, N_HIDDEN, HID], BF16, kind="ExternalInput")
    wo_d = nc.dram_tensor("wo", [HID, 1], BF16, kind="ExternalInput")
    bia_d = nc.dram_tensor("bia", [HID, 5], F32, kind="ExternalInput")
    bo_d = nc.dram_tensor("bo", [1], F32, kind="ExternalInput")
    out_d = nc.dram_tensor("out", [b_core, N_NODES], F32, kind="ExternalOutput")

    n_chunks = b_core // chunk_b
    chunk_tok = chunk_b * N_NODES
    tiles = chunk_tok // N_TILE  # matmul tiles per layer sweep (16)
    pairs = tiles // 2
    quads = tiles // 4

    Relu = mybir.ActivationFunctionType.Relu
    Identity = mybir.ActivationFunctionType.Identity
    ALU_ADD = mybir.AluOpType.add
    ALU_MAX = mybir.AluOpType.max

    eng_t = [0.0, 0.0]  # modeled busy ns: [ACT, DVE]

    with _tile.TileContext(nc) as tc:
        with (
            tc.tile_pool(name="const", bufs=1) as const,
            tc.tile_pool(name="xin", bufs=1) as xin,
            tc.tile_pool(name="outp", bufs=2) as outp,
            tc.tile_pool(name="hbuf", bufs=2 * pairs) as hbuf,
            tc.tile_pool(name="ps", bufs=4, space="PSUM") as ps,
        ):
            # ---- constants ----
            # the first matmul needs x chunk 0 + wz + ctrl rows: issue those
            # DMAs first so they are never queued behind the bulky consts.
            xt = xin.tile([7, chunk_tok], BF16, tag="xt")
            nc.sync.dma_start(
                xt[0:4, :], xs_d[:, 0:chunk_tok]
            )
            wz = const.tile([7, HID], BF16)
            nc.sync.dma_start(wz[:], wz_d[:])
            nc.sync.dma_start(xt[4:7, :], ctl_d[:])
            bia = const.tile([HID, 5], F32)
            nc.sync.dma_start(bia[:], bia_d[:])
            bin_t = bia  # column 0 = b_in, columns 1..4 = b_h
            wh = const.tile([HID, N_HIDDEN, HID], BF16)
            nc.sync.dma_start(wh[:], wh_d[:])
            wo = const.tile([HID, 1], BF16)
            nc.sync.dma_start(wo[:], wo_d[:])
            bo_t = const.tile([1, 1], F32)
            nc.sync.dma_start(bo_t[:], bo_d[:].unsqueeze(0))

            def drain(dst, src, bias_ap, relu, kind, eng=None):
                """PSUM->SBUF bias+(relu). eng forces ACT(0)/DVE(1); default
                picks the modeled-least-busy engine."""
                ca, cd = _COST[f"act_{kind}"], _COST[f"dve_{kind}"]
                if eng is None:
                    use_act = eng_t[0] + ca <= eng_t[1] + cd
                else:
                    use_act = eng == 0
                if use_act:
                    eng_t[0] += ca
                    nc.scalar.activation(
                        dst, src, Relu if relu else Identity, bias=bias_ap
                    )
                elif relu:
                    eng_t[1] += cd
                    nc.vector.tensor_scalar(
                        out=dst,
                        in0=src,
                        scalar1=bias_ap,
                        scalar2=0.0,
                        op0=ALU_ADD,
                        op1=ALU_MAX,
                    )
                else:
                    eng_t[1] += cd
                    nc.vector.tensor_scalar(
                        out=dst,
                        in0=src,
                        scalar1=bias_ap,
                        scalar2=None,
                        op0=ALU_ADD,
                    )

            for ck in range(n_chunks):
                b0 = ck * chunk_b
                t0 = b0 * N_NODES
                if ck > 0:
                    nc.sync.dma_start(
                        xt[0:4, :], xs_d[:, t0 : t0 + chunk_tok]
                    )

                # ---- layer 0 sweep: one K=7 matmul per tile
                hs = []
                for pr in range(pairs):
                    pt = ps.tile([HID, 2 * N_TILE], F32, tag="ps")
                    for half in range(2):
                        c0 = (2 * pr + half) * N_TILE
                        nc.tensor.matmul(
                            pt[:, half * N_TILE : (half + 1) * N_TILE],
                            wz[:],
                            xt[:, c0 : c0 + N_TILE],
                        )
                    h = hbuf.tile([HID, 2 * N_TILE], BF16, tag="h")
                    drain(h[:], pt[:], bia[:, 0:1], relu=True, kind="pair")
                    hs.append(h)

                # ---- hidden layer sweeps (out matmuls interleave into l3)
                for li in range(N_HIDDEN):
                    last = li == N_HIDDEN - 1
                    if last:
                        o = outp.tile([1, chunk_tok], F32, tag="o")
                        emitted = set()
                    hs2 = []
                    for pr in range(pairs):
                        pt = ps.tile([HID, 2 * N_TILE], F32, tag="ps")
                        for half in range(2):
                            nc.tensor.matmul(
                                pt[:, half * N_TILE : (half + 1) * N_TILE],
                                wh[:, li, :],
                                hs[pr][:, half * N_TILE : (half + 1) * N_TILE],
                            )
                        h = hbuf.tile([HID, 2 * N_TILE], BF16, tag="h")
                        drain(
                            h[:],
                            pt[:],
                            bia[:, li + 1 : li + 2],
                            relu=True,
                            kind="pair",
                        )
                        hs2.append(h)
                        if last and pr >= 2:
                            # out matmuls for the pair drained 2 pairs ago
                            _emit_out_pair(
                                nc, ps, hs2, o, bo_t, wo, pr - 2, drain
                            )
                            emitted.add(pr - 2)
                    hs = hs2
                    if last:
                        for pr in range(pairs):
                            if pr not in emitted:
                                _emit_out_pair(nc, ps, hs, o, bo_t, wo, pr, drain)

                nc.sync.dma_start(
                    out_d[b0 : b0 + chunk_b].rearrange("b n -> (b n)").unsqueeze(0),
                    o[:],
                )

    _split_excess_sync_waits(nc)
    return nc


def _emit_out_pair(nc, ps, hs, o, bo_t, wo, pr, drain, eng=None):
    """2 out matmuls (M=1) into partition 0 of the bank-aligned halves of
    a pool slot shared with the hidden-layer pairs (one unified 4-deep
    PSUM ring), retired by a single [1,1024] drain."""
    pt = ps.tile([HID, 2 * N_TILE], mybir.dt.float32, tag="ps")
    for half in range(2):
        nc.tensor.matmul(
            pt[0:1, half * N_TILE : (half + 1) * N_TILE],
            wo[:],
            hs[pr][:, half * N_TILE : (half + 1) * N_TILE],
        )
    drain(
        o[0:1, pr * 2 * N_TILE : (pr + 1) * 2 * N_TILE],
        pt[0:1, :],
        bo_t[0:1, 0:1],
        relu=False,
        kind="pair",
        eng=eng,
    )


def _prep_weights(controls, W_in, b_in, W_h, b_h, W_out, b_out, data_idx, ctrl_idx):
    """Host-side prep: gather W_in rows per electrode placement (replicating
    the reference's scatter semantics) and tile the control matrix to one
    chunk."""
    di = np.asarray(data_idx)[0].tolist()  # placements identical across nodes
    ci = np.asarray(ctrl_idx)[0].tolist()
    W_in = np.asarray(W_in, dtype=np.float32)
    Wd = W_in[di, :].copy()  # [4, HID]
    cset = set(ci)
    for j in range(4):
        if di[j] in cset or di[j] in di[j + 1 :]:
            Wd[j] = 0.0  # overwritten by a control (or a later data) electrode
    Wc = W_in[ci, :].copy()  # [3, HID]
    for k in range(3):
        if ci[k] in ci[k + 1 :]:
            Wc[k] = 0.0  # later control write wins

    bf = ml_dtypes.bfloat16
    ctl = np.ascontiguousarray(
        np.tile(np.asarray(controls, np.float32).T, (1, CHUNK_B)).astype(bf)
    )  # [3, chunk_tok]

    common = {
        "ctl": ctl,
        "wz": np.ascontiguousarray(
            np.concatenate([Wd, Wc], axis=0).astype(bf)
        ),  # [7, HID]
        "wh": np.ascontiguousarray(
            np.asarray(W_h, np.float32).astype(bf).transpose(1, 0, 2)
        ),
        "wo": np.ascontiguousarray(np.asarray(W_out, np.float32).astype(bf)),
        "bia": np.ascontiguousarray(
            np.concatenate(
                [np.asarray(b_in, np.float32)[:, None],
                 np.asarray(b_h, np.float32).T],
                axis=1,
            )
        ),
        "bo": np.ascontiguousarray(np.asarray(b_out, np.float32)),
    }
    return common


def _shuffle_x(x_core):
    """[b,64,64] -> [4, b*1024]: partition p=(kh*2+kw), tokens (b, node)."""
    b = x_core.shape[0]
    p = x_core.reshape(b, 32, 2, 32, 2).transpose(2, 4, 0, 1, 3)
    return np.ascontiguousarray(
        p.reshape(4, b * N_NODES).astype(ml_dtypes.bfloat16)
    )


def _run(inputs, trace=False, tmpdir=None):
    x = np.asarray(inputs["x"], dtype=np.float32)
    common = _prep_weights(
        inputs["controls"],
        inputs["W_in"],
        inputs["b_in"],
        inputs["W_h"],
        inputs["b_h"],
        inputs["W_out"],
        inputs["b_out"],
        inputs["data_idx"],
        inputs["ctrl_idx"],
    )

    nc = _build_program(B_CORE, CHUNK_B)

    core_ids = list(range(N_CORES))
    in_maps = []
    for i in core_ids:
        shard = _shuffle_x(x[i * B_CORE : (i + 1) * B_CORE, 0])
        in_maps.append({"xs": shard, **common})

    res = run_bass_kernel_spmd(nc, in_maps, core_ids, trace=trace, tmpdir=tmpdir)
    out = np.concatenate([res.results[i]["out"] for i in core_ids], axis=0)
    return out.reshape(B, 32, 32), res.exec_time_ns


def kernel(**inputs):
    return _run(inputs, trace=False)[0]
